# revision 12
# baseline (speedup 1.0000x reference)
"""Inverse 2x2 Haar wavelet transform on 8 Trainium2 NeuronCores.

Full inputs:  ll (16, 64, 128, 128) f32, hf (16, 192, 128, 128) f32
Full output:  (16, 64, 256, 256) f32

Sharding: pure data-parallel over batch; core i gets batches [2i, 2i+2).

Per-core kernel: raw Bass 4-engine pipeline with variable-size channel
groups and both HWDGE rings carrying a mix of input and output DMAs.

  SP/ACT  each issue input DMAs (ll + 3 hf subbands) for alternating
          groups and the output DMA for the opposite groups, so the two
          FIFO HWDGE rings stay balanced (in-bytes == out-bytes per
          group makes a dedicated-ring split structurally lag by one
          full group of input + compute),
  DVE     runs the butterfly (t1=ll-lh, t2=hl-hh, s1=ll+lh, s2=hl+hh ->
          a,b,c,d) on rows [0, rd) of each group,
  Pool    (GPSIMD) does the same on rows [rd, G) with its own temp
          tiles (~2.2 ns/elem vs DVE ~1.04, so it gets the smaller
          share); compute then never gates the DMA streams.

Group sizes ramp up geometrically (2,3,4,6,9,12,...) so the pipeline
fill before the first output DMA is small, and the last groups are
small so the final compute+store drain is short.

Tile layout: partition p of a group's tile holds G*H/128 consecutive
rows of the flat (G*H, W) row space, so input DMAs are >=1KB-per-
partition bursts and the output DMA is fully contiguous per partition.

Raw semaphores (not Tile) because TRN2 instructions hold at most one
sync-wait; standalone wait_ge instructions sidestep that cap.
"""

import os
import sys

import numpy as np

# Make concourse importable in a bare environment without shadowing the
# ambient PYTHONPATH (the axon jax plugin lives in /root/.axon_site).
for _p in (
    "/root/.axon_site",
    "/root/.axon_site/_ro/trn_rl_repo",
    "/root/.axon_site/_ro/pypackages",
    "/opt/trn_rl_repo",
):
    if _p not in sys.path and os.path.isdir(_p):
        sys.path.append(_p)

from concourse import bass, mybir
from concourse.bass_utils import run_bass_kernel_spmd

N_CORES = 8
B, C, H, W = 16, 64, 128, 128
B_LOC = B // N_CORES

# Channel-group sizes per batch (each batch's list sums to C). First
# batch ramps up (short pipeline fill), last batch tapers (short drain).
SCHED_FIRST = [2, 2, 4, 8, 16, 16, 16]
SCHED_LAST = [16, 16, 16, 8, 4, 4]
# DVE rows per group size; Pool (GPSIMD) takes the rest (ratio ~ DVE:Pool
# element rates 1.04 : 2.2 ns/elem), with small groups DVE-only.
DVE_ROWS = {16: 10, 8: 6, 4: 4, 2: 2}
NBUF_IN = 2
NBUF_OUT = 3
MIX_RINGS = True


def _schedule(B_loc, C):
    groups = []
    for b in range(B_loc):
        sizes = SCHED_FIRST if b == 0 else (SCHED_LAST if b == B_loc - 1 else None)
        if sizes is None:
            sizes = [16, 16, 16, 16]  # interior batches
        assert sum(sizes) == C
        c0 = 0
        for G in sizes:
            groups.append((b, c0, G))
            c0 += G
    return groups


def build_haar_nc(B_loc=B_LOC, C=C, H=H, W=W, nbuf=NBUF_IN, nbuf_out=NBUF_OUT, mix=MIX_RINGS):
    P = 128
    assert H == P
    dt = mybir.dt.float32
    sub = mybir.AluOpType.subtract
    add = mybir.AluOpType.add

    nc = bass.Bass()
    ll_ext = nc.dram_tensor("ll", [B_loc, C, H, W], dt, kind="ExternalInput")
    hf_ext = nc.dram_tensor("hf", [B_loc, 3 * C, H, W], dt, kind="ExternalInput")
    out_ext = nc.dram_tensor("out", [B_loc, C, 2 * H, 2 * W], dt, kind="ExternalOutput")

    groups = _schedule(B_loc, C)
    NG = len(groups)
    G_MAX = max(G for _, _, G in groups)
    rd_of = [DVE_ROWS[G] for _, _, G in groups]
    pr_of = [G - rd for (_, _, G), rd in zip(groups, rd_of)]
    pool_ops = [8 if pr else 0 for pr in pr_of]
    cum_pool, acc = [], 0
    for g in range(NG):
        acc += pool_ops[g]
        cum_pool.append(acc)
    pool_base = [cum_pool[g] - pool_ops[g] for g in range(NG)]

    # (C, 3, H, W) DRAM view of each batch's stacked subbands
    hf4 = [hf_ext[b].rearrange("(c s) h w -> c s h w", s=3) for b in range(B_loc)]

    from contextlib import ExitStack

    with ExitStack() as ctx:
        block = ctx.enter_context(nc.Block())
        # Per-buffer-slot DMA sems: completions of different DMAs are
        # unordered, so a single cumulative counter could reach a group's
        # threshold while one of that group's DMAs is still in flight.
        # Same-slot groups ARE ordered (slot reuse waits on compute/flush),
        # so per-slot cumulative thresholds are exact.
        s_ina = [ctx.enter_context(nc.semaphore(f"s_ina{i}")) for i in range(nbuf)]
        s_inb = [ctx.enter_context(nc.semaphore(f"s_inb{i}")) for i in range(nbuf)]
        s_dve = ctx.enter_context(nc.semaphore("s_dve"))
        s_pool = ctx.enter_context(nc.semaphore("s_pool"))
        s_out = [ctx.enter_context(nc.semaphore(f"s_out{i}")) for i in range(nbuf_out)]

        LLb, HFb, OUTb = [], [], []
        for i in range(nbuf):
            LLb.append(ctx.enter_context(nc.sbuf_tensor(f"LL{i}", [P, G_MAX, W], dt)))
            HFb.append(
                ctx.enter_context(nc.sbuf_tensor(f"HF{i}", [P, 3, G_MAX, W], dt))
            )
        for i in range(nbuf_out):
            OUTb.append(
                ctx.enter_context(nc.sbuf_tensor(f"OUT{i}", [P, G_MAX, 2, W, 2], dt))
            )
        # Per-engine temp tiles: no cross-engine WAR hazards on temps.
        RD_MAX = max(rd_of)
        PR_MAX = max(pr_of)
        T1d = ctx.enter_context(nc.sbuf_tensor("T1d", [P, RD_MAX, W], dt))
        T2d = ctx.enter_context(nc.sbuf_tensor("T2d", [P, RD_MAX, W], dt))
        S1d = ctx.enter_context(nc.sbuf_tensor("S1d", [P, RD_MAX, W], dt))
        S2d = ctx.enter_context(nc.sbuf_tensor("S2d", [P, RD_MAX, W], dt))
        T1p = ctx.enter_context(nc.sbuf_tensor("T1p", [P, PR_MAX, W], dt))
        T2p = ctx.enter_context(nc.sbuf_tensor("T2p", [P, PR_MAX, W], dt))
        S1p = ctx.enter_context(nc.sbuf_tensor("S1p", [P, PR_MAX, W], dt))
        S2p = ctx.enter_context(nc.sbuf_tensor("S2p", [P, PR_MAX, W], dt))

        # Per-group DMA work, split across both HWDGE rings:
        #   in_a[g] = ll + lh   (ring g%2)
        #   in_b[g] = hl + hh   (ring 1-g%2)      -> input latency halved
        #   out_d[g] = OUT rows [0, rd)  (gated on s_dve only,  ring 1-g%2)
        #   out_p[g] = OUT rows [rd, G)  (gated on s_pool only, ring g%2)
        # Per-2-groups each ring moves in_a+in_b+out_d+out_p = 8 MiB-equiv,
        # so the rings stay byte-balanced.
        out_dmas = [1 if pool_ops[g] == 0 else 2 for g in range(NG)]
        out_cum_after = [0] * NG
        _cum_slot = {}
        for g in range(NG):
            io = g % nbuf_out
            _cum_slot[io] = _cum_slot.get(io, 0) + 16 * out_dmas[g]
            out_cum_after[g] = _cum_slot[io]

        # flat DRAM views of each group's output block: [128, 4*G*W] with
        # partition p holding its contiguous 4*G*W chunk (matches OUT tile)
        def out_flat(b, c0, G):
            return (
                out_ext[b, c0 : c0 + G]
                .rearrange("c h w -> (c h w)")
                .rearrange("(p f) -> p f", p=P)
            )

        LOOKAHEAD = 2
        events = []
        for g in range(NG):
            events.append(("in", g))
            if g >= LOOKAHEAD:
                events.append(("out", g - LOOKAHEAD))
        for g in range(NG - LOOKAHEAD, NG):
            events.append(("out", g))

        def emit_ring(eng, ring):
            for kind, g in events:
                b, c0, G = groups[g]
                i = g % nbuf
                io = g % nbuf_out
                rd = rd_of[g]
                if kind == "in":
                    half = "a" if g % 2 == ring else "b"
                    if g >= nbuf:
                        pg = g - nbuf
                        # slot free once the ops reading this half's
                        # tiles (T1/S1 for ll+lh, T2/S2 for hl+hh) of
                        # group pg are done on both engines
                        k = 2 if half == "a" else 4
                        eng.wait_ge(s_dve, 8 * pg + k)
                        if pool_ops[pg]:
                            eng.wait_ge(s_pool, pool_base[pg] + k)
                    if half == "a":
                        eng.dma_start(
                            out=LLb[i][:, :G], in_=ll_ext[b, c0 : c0 + G]
                        ).then_inc(s_ina[i], 16)
                        eng.dma_start(
                            out=HFb[i][:, 0, :G], in_=hf4[b][c0 : c0 + G, 0]
                        ).then_inc(s_ina[i], 16)
                    else:
                        for s in (1, 2):
                            eng.dma_start(
                                out=HFb[i][:, s, :G], in_=hf4[b][c0 : c0 + G, s]
                            ).then_inc(s_inb[i], 16)
                else:
                    fv = out_flat(b, c0, G)
                    if pool_ops[g] == 0:
                        if (1 - g % 2) != ring:
                            continue
                        eng.wait_ge(s_dve, 8 * (g + 1))
                        eng.dma_start(
                            out=out_ext[b, c0 : c0 + G], in_=OUTb[io][:, :G]
                        ).then_inc(s_out[io], 16)
                    elif (1 - g % 2) == ring:
                        # DVE rows [0, rd): per-partition prefix
                        eng.wait_ge(s_dve, 8 * (g + 1))
                        eng.dma_start(
                            out=fv[:, : 4 * rd * W], in_=OUTb[io][:, :rd]
                        ).then_inc(s_out[io], 16)
                    else:
                        # Pool rows [rd, G): per-partition suffix
                        eng.wait_ge(s_pool, cum_pool[g])
                        eng.dma_start(
                            out=fv[:, 4 * rd * W :], in_=OUTb[io][:, rd:G]
                        ).then_inc(s_out[io], 16)

        @block.vector
        def _(vector: bass.BassEngine):
            for g, (b, c0, G) in enumerate(groups):
                i = g % nbuf
                rd = rd_of[g]
                # ll+lh landed: T1/S1 can start before hl/hh arrive
                vector.wait_ge(s_ina[i], 32 * (g // nbuf + 1))
                io = g % nbuf_out
                LL, HF, OUT = LLb[i], HFb[i], OUTb[io]
                ll_v = LL[:, :rd]
                lh_v, hl_v, hh_v = HF[:, 0, :rd], HF[:, 1, :rd], HF[:, 2, :rd]
                t1, t2 = T1d[:, :rd], T2d[:, :rd]
                s1, s2 = S1d[:, :rd], S2d[:, :rd]
                if g >= 1:
                    # WAR: prev group's a,b ops still read T1d,T2d
                    vector.wait_ge(s_dve, 8 * g - 2)
                vector.tensor_tensor(t1, ll_v, lh_v, sub).then_inc(s_dve, 1)
                if g >= 1:
                    # WAR: prev group's c,d ops still read S1d,S2d
                    vector.wait_ge(s_dve, 8 * g)
                vector.tensor_tensor(s1, ll_v, lh_v, add).then_inc(s_dve, 1)
                vector.wait_ge(s_inb[i], 32 * (g // nbuf + 1))
                vector.tensor_tensor(t2, hl_v, hh_v, sub).then_inc(s_dve, 1)
                vector.tensor_tensor(s2, hl_v, hh_v, add).then_inc(s_dve, 1)
                # DVE has no internal RAW interlock: wait for our own
                # completions before consuming temp tiles.
                vector.wait_ge(s_dve, 8 * g + 3)
                if g >= nbuf_out:
                    # OUT slot flushed (group g-nbuf_out stored)
                    vector.wait_ge(s_out[io], out_cum_after[g - nbuf_out])
                vector.tensor_tensor(OUT[:, :rd, 0, :, 0], t1, t2, sub).then_inc(
                    s_dve, 1
                )
                vector.tensor_tensor(OUT[:, :rd, 0, :, 1], t1, t2, add).then_inc(
                    s_dve, 1
                )
                vector.wait_ge(s_dve, 8 * g + 4)
                vector.tensor_tensor(OUT[:, :rd, 1, :, 0], s1, s2, sub).then_inc(
                    s_dve, 1
                )
                vector.tensor_tensor(OUT[:, :rd, 1, :, 1], s1, s2, add).then_inc(
                    s_dve, 1
                )

        @block.gpsimd
        def _(pool: bass.BassEngine):
            for g, (b, c0, G) in enumerate(groups):
                if not pool_ops[g]:
                    continue
                i = g % nbuf
                rd, pr = rd_of[g], pr_of[g]
                base = pool_base[g]
                pool.wait_ge(s_ina[i], 32 * (g // nbuf + 1))
                io = g % nbuf_out
                LL, HF, OUT = LLb[i], HFb[i], OUTb[io]
                ll_v = LL[:, rd:G]
                lh_v, hl_v, hh_v = HF[:, 0, rd:G], HF[:, 1, rd:G], HF[:, 2, rd:G]
                t1, t2 = T1p[:, :pr], T2p[:, :pr]
                s1, s2 = S1p[:, :pr], S2p[:, :pr]
                if base >= 8:
                    # WAR: prev active group's a,b ops still read T1p,T2p
                    pool.wait_ge(s_pool, base - 2)
                pool.tensor_tensor(t1, ll_v, lh_v, sub).then_inc(s_pool, 1)
                if base >= 8:
                    pool.wait_ge(s_pool, base)
                pool.tensor_tensor(s1, ll_v, lh_v, add).then_inc(s_pool, 1)
                pool.wait_ge(s_inb[i], 32 * (g // nbuf + 1))
                pool.tensor_tensor(t2, hl_v, hh_v, sub).then_inc(s_pool, 1)
                pool.tensor_tensor(s2, hl_v, hh_v, add).then_inc(s_pool, 1)
                pool.wait_ge(s_pool, base + 3)
                if g >= nbuf_out:
                    pool.wait_ge(s_out[io], out_cum_after[g - nbuf_out])
                pool.tensor_tensor(OUT[:, rd:G, 0, :, 0], t1, t2, sub).then_inc(
                    s_pool, 1
                )
                pool.tensor_tensor(OUT[:, rd:G, 0, :, 1], t1, t2, add).then_inc(
                    s_pool, 1
                )
                pool.wait_ge(s_pool, base + 4)
                pool.tensor_tensor(OUT[:, rd:G, 1, :, 0], s1, s2, sub).then_inc(
                    s_pool, 1
                )
                pool.tensor_tensor(OUT[:, rd:G, 1, :, 1], s1, s2, add).then_inc(
                    s_pool, 1
                )

    return nc


_NC_CACHE = {}


def _get_nc():
    if "nc" not in _NC_CACHE:
        _NC_CACHE["nc"] = build_haar_nc()
    return _NC_CACHE["nc"]


def kernel(ll: np.ndarray, hf: np.ndarray) -> np.ndarray:
    ll = np.ascontiguousarray(ll, dtype=np.float32)
    hf = np.ascontiguousarray(hf, dtype=np.float32)
    nc = _get_nc()
    in_maps = [
        {
            "ll": ll[i * B_LOC : (i + 1) * B_LOC],
            "hf": hf[i * B_LOC : (i + 1) * B_LOC],
        }
        for i in range(N_CORES)
    ]
    res = run_bass_kernel_spmd(nc, in_maps, list(range(N_CORES))).results
    return np.concatenate([res[i]["out"] for i in range(N_CORES)], axis=0)


# revision 18
# speedup vs baseline: 2.1784x; 2.1784x over previous
"""Inverse 2x2 Haar wavelet transform on 8 Trainium2 NeuronCores.

Full inputs:  ll (16, 64, 128, 128) f32, hf (16, 192, 128, 128) f32
Full output:  (16, 64, 256, 256) f32

Sharding: pure data-parallel over batch; core i gets batches [2i, 2i+2).

Per-core kernel: raw Bass 4-engine pipeline with variable-size channel
groups and both HWDGE rings carrying a mix of input and output DMAs.

  SP/ACT  each carry half of every group's input (ll+lh on one ring,
          hl+hh on the other: input latency halved) plus one output
          half-flush (DVE's rows gated on s_dve alone, Pool's rows on
          s_pool alone), so the two FIFO HWDGE rings stay byte-balanced
          and neither direction structurally lags,
  DVE     runs the butterfly (t1=ll-lh, t2=hl-hh, s1=ll+lh, s2=hl+hh ->
          a,b,c,d) on rows [0, rd) of each group,
  Pool    (GPSIMD) does the same on rows [rd, G) with its own temp
          tiles (~2.2 ns/elem vs DVE ~1.04, so it gets the smaller
          share); compute then never gates the DMA streams.

Group sizes ramp up (4,4,8,16,...) so the pipeline fill before the
first output DMA is small, and the last groups taper (...,8,4,4) so
the final compute+store drain is short.

Tile layout: partition p of a group's tile holds G*H/128 consecutive
rows of the flat (G*H, W) row space, so input DMAs are >=1KB-per-
partition bursts and the output DMA is fully contiguous per partition.

Raw semaphores (not Tile) because TRN2 instructions hold at most one
sync-wait; standalone wait_ge instructions sidestep that cap.
"""

import os
import sys

import numpy as np

# Make concourse importable in a bare environment without shadowing the
# ambient PYTHONPATH (the axon jax plugin lives in /root/.axon_site).
for _p in (
    "/root/.axon_site",
    "/root/.axon_site/_ro/trn_rl_repo",
    "/root/.axon_site/_ro/pypackages",
    "/opt/trn_rl_repo",
):
    if _p not in sys.path and os.path.isdir(_p):
        sys.path.append(_p)

from concourse import bass, mybir
from concourse.bass_utils import run_bass_kernel_spmd

N_CORES = 8
B, C, H, W = 16, 64, 128, 128
B_LOC = B // N_CORES

# Channel-group sizes per batch (each batch's list sums to C). First
# batch ramps up (short pipeline fill), last batch tapers (short drain).
SCHED_FIRST = [4, 4, 8, 16, 16, 16]
SCHED_LAST = [16, 16, 16, 8, 4, 4]
# DVE rows per group size; Pool (GPSIMD) takes the rest (ratio ~ DVE:Pool
# element rates 1.04 : 2.2 ns/elem), with small groups DVE-only.
DVE_ROWS = {16: 10, 8: 6, 4: 4, 2: 2}
NBUF_IN = 2
NBUF_OUT = 3
MIX_RINGS = True


def _schedule(B_loc, C):
    groups = []
    for b in range(B_loc):
        sizes = SCHED_FIRST if b == 0 else (SCHED_LAST if b == B_loc - 1 else None)
        if sizes is None:
            sizes = [16, 16, 16, 16]  # interior batches
        assert sum(sizes) == C
        c0 = 0
        for G in sizes:
            groups.append((b, c0, G))
            c0 += G
    return groups


def build_haar_nc(B_loc=B_LOC, C=C, H=H, W=W, nbuf=NBUF_IN, nbuf_out=NBUF_OUT, mix=MIX_RINGS):
    P = 128
    assert H == P
    dt = mybir.dt.float32
    sub = mybir.AluOpType.subtract
    add = mybir.AluOpType.add

    nc = bass.Bass()
    ll_ext = nc.dram_tensor("ll", [B_loc, C, H, W], dt, kind="ExternalInput")
    hf_ext = nc.dram_tensor("hf", [B_loc, 3 * C, H, W], dt, kind="ExternalInput")
    out_ext = nc.dram_tensor("out", [B_loc, C, 2 * H, 2 * W], dt, kind="ExternalOutput")

    groups = _schedule(B_loc, C)
    NG = len(groups)
    G_MAX = max(G for _, _, G in groups)
    rd_of = [DVE_ROWS[G] for _, _, G in groups]
    pr_of = [G - rd for (_, _, G), rd in zip(groups, rd_of)]
    pool_ops = [8 if pr else 0 for pr in pr_of]
    cum_pool, acc = [], 0
    for g in range(NG):
        acc += pool_ops[g]
        cum_pool.append(acc)
    pool_base = [cum_pool[g] - pool_ops[g] for g in range(NG)]

    # (C, 3, H, W) DRAM view of each batch's stacked subbands
    hf4 = [hf_ext[b].rearrange("(c s) h w -> c s h w", s=3) for b in range(B_loc)]

    from contextlib import ExitStack

    with ExitStack() as ctx:
        block = ctx.enter_context(nc.Block())
        # Per-buffer-slot DMA sems: completions of different DMAs are
        # unordered, so a single cumulative counter could reach a group's
        # threshold while one of that group's DMAs is still in flight.
        # Same-slot groups ARE ordered (slot reuse waits on compute/flush),
        # so per-slot cumulative thresholds are exact.
        s_ina = [ctx.enter_context(nc.semaphore(f"s_ina{i}")) for i in range(nbuf)]
        s_inb = [ctx.enter_context(nc.semaphore(f"s_inb{i}")) for i in range(nbuf)]
        s_dve = ctx.enter_context(nc.semaphore("s_dve"))
        s_pool = ctx.enter_context(nc.semaphore("s_pool"))
        s_out = [ctx.enter_context(nc.semaphore(f"s_out{i}")) for i in range(nbuf_out)]

        LLb, HFb, OUTb = [], [], []
        for i in range(nbuf):
            LLb.append(ctx.enter_context(nc.sbuf_tensor(f"LL{i}", [P, G_MAX, W], dt)))
            HFb.append(
                ctx.enter_context(nc.sbuf_tensor(f"HF{i}", [P, 3, G_MAX, W], dt))
            )
        for i in range(nbuf_out):
            OUTb.append(
                ctx.enter_context(nc.sbuf_tensor(f"OUT{i}", [P, G_MAX, 2, W, 2], dt))
            )
        # Per-engine temp tiles: no cross-engine WAR hazards on temps.
        RD_MAX = max(rd_of)
        PR_MAX = max(pr_of)
        T1d = ctx.enter_context(nc.sbuf_tensor("T1d", [P, RD_MAX, W], dt))
        T2d = ctx.enter_context(nc.sbuf_tensor("T2d", [P, RD_MAX, W], dt))
        S1d = ctx.enter_context(nc.sbuf_tensor("S1d", [P, RD_MAX, W], dt))
        S2d = ctx.enter_context(nc.sbuf_tensor("S2d", [P, RD_MAX, W], dt))
        T1p = ctx.enter_context(nc.sbuf_tensor("T1p", [P, PR_MAX, W], dt))
        T2p = ctx.enter_context(nc.sbuf_tensor("T2p", [P, PR_MAX, W], dt))
        S1p = ctx.enter_context(nc.sbuf_tensor("S1p", [P, PR_MAX, W], dt))
        S2p = ctx.enter_context(nc.sbuf_tensor("S2p", [P, PR_MAX, W], dt))

        # Per-group DMA work, split across both HWDGE rings:
        #   in_a[g] = ll + lh   (ring g%2)
        #   in_b[g] = hl + hh   (ring 1-g%2)      -> input latency halved
        #   out_d[g] = OUT rows [0, rd)  (gated on s_dve only,  ring 1-g%2)
        #   out_p[g] = OUT rows [rd, G)  (gated on s_pool only, ring g%2)
        # Per-2-groups each ring moves in_a+in_b+out_d+out_p = 8 MiB-equiv,
        # so the rings stay byte-balanced.
        out_dmas = [1 if pool_ops[g] == 0 else 2 for g in range(NG)]
        out_cum_after = [0] * NG
        _cum_slot = {}
        for g in range(NG):
            io = g % nbuf_out
            _cum_slot[io] = _cum_slot.get(io, 0) + 16 * out_dmas[g]
            out_cum_after[g] = _cum_slot[io]

        # flat DRAM views of each group's output block: [128, 4*G*W] with
        # partition p holding its contiguous 4*G*W chunk (matches OUT tile)
        def out_flat(b, c0, G):
            return (
                out_ext[b, c0 : c0 + G]
                .rearrange("c h w -> (c h w)")
                .rearrange("(p f) -> p f", p=P)
            )

        LOOKAHEAD = 2
        events = []
        for g in range(NG):
            events.append(("in", g))
            if g >= LOOKAHEAD:
                events.append(("out", g - LOOKAHEAD))
        for g in range(NG - LOOKAHEAD, NG):
            events.append(("out", g))

        # ring parity per group: in_a & out_p ride ring pa[g], in_b &
        # out_d ride the other; chosen to balance ring byte loads.
        pa = [(g // 2) % 2 for g in range(NG)]

        def emit_ring(eng, ring):
            for kind, g in events:
                b, c0, G = groups[g]
                i = g % nbuf
                io = g % nbuf_out
                rd = rd_of[g]
                if kind == "in":
                    half = "a" if pa[g] == ring else "b"
                    if g >= nbuf:
                        pg = g - nbuf
                        # slot free once the ops reading this half's
                        # tiles (T1/S1 for ll+lh, T2/S2 for hl+hh) of
                        # group pg are done on both engines
                        k = 2 if half == "a" else 4
                        eng.wait_ge(s_dve, 8 * pg + k)
                        if pool_ops[pg]:
                            eng.wait_ge(s_pool, pool_base[pg] + k)
                    if half == "a":
                        eng.dma_start(
                            out=LLb[i][:, :G], in_=ll_ext[b, c0 : c0 + G]
                        ).then_inc(s_ina[i], 16)
                        eng.dma_start(
                            out=HFb[i][:, 0, :G], in_=hf4[b][c0 : c0 + G, 0]
                        ).then_inc(s_ina[i], 16)
                    else:
                        for s in (1, 2):
                            eng.dma_start(
                                out=HFb[i][:, s, :G], in_=hf4[b][c0 : c0 + G, s]
                            ).then_inc(s_inb[i], 16)
                else:
                    fv = out_flat(b, c0, G)
                    if pool_ops[g] == 0:
                        if (1 - pa[g]) != ring:
                            continue
                        eng.wait_ge(s_dve, 8 * (g + 1))
                        eng.dma_start(
                            out=out_ext[b, c0 : c0 + G], in_=OUTb[io][:, :G]
                        ).then_inc(s_out[io], 16)
                    elif (1 - pa[g]) == ring:
                        # DVE rows [0, rd): per-partition prefix
                        eng.wait_ge(s_dve, 8 * (g + 1))
                        eng.dma_start(
                            out=fv[:, : 4 * rd * W], in_=OUTb[io][:, :rd]
                        ).then_inc(s_out[io], 16)
                    else:
                        # Pool rows [rd, G): per-partition suffix
                        eng.wait_ge(s_pool, cum_pool[g])
                        eng.dma_start(
                            out=fv[:, 4 * rd * W :], in_=OUTb[io][:, rd:G]
                        ).then_inc(s_out[io], 16)

        @block.sync
        def _(sync: bass.BassEngine):
            emit_ring(sync, 0)

        @block.scalar
        def _(scalar: bass.BassEngine):
            emit_ring(scalar, 1)

        @block.vector
        def _(vector: bass.BassEngine):
            for g, (b, c0, G) in enumerate(groups):
                i = g % nbuf
                rd = rd_of[g]
                # ll+lh landed: T1/S1 can start before hl/hh arrive
                vector.wait_ge(s_ina[i], 32 * (g // nbuf + 1))
                io = g % nbuf_out
                LL, HF, OUT = LLb[i], HFb[i], OUTb[io]
                ll_v = LL[:, :rd]
                lh_v, hl_v, hh_v = HF[:, 0, :rd], HF[:, 1, :rd], HF[:, 2, :rd]
                t1, t2 = T1d[:, :rd], T2d[:, :rd]
                s1, s2 = S1d[:, :rd], S2d[:, :rd]
                if g >= 1:
                    # WAR: prev group's a,b ops still read T1d,T2d
                    vector.wait_ge(s_dve, 8 * g - 2)
                vector.tensor_tensor(t1, ll_v, lh_v, sub).then_inc(s_dve, 1)
                if g >= 1:
                    # WAR: prev group's c,d ops still read S1d,S2d
                    vector.wait_ge(s_dve, 8 * g)
                vector.tensor_tensor(s1, ll_v, lh_v, add).then_inc(s_dve, 1)
                vector.wait_ge(s_inb[i], 32 * (g // nbuf + 1))
                vector.tensor_tensor(t2, hl_v, hh_v, sub).then_inc(s_dve, 1)
                vector.tensor_tensor(s2, hl_v, hh_v, add).then_inc(s_dve, 1)
                # DVE has no internal RAW interlock: wait for our own
                # completions before consuming temp tiles.
                vector.wait_ge(s_dve, 8 * g + 3)
                if g >= nbuf_out:
                    # OUT slot flushed (group g-nbuf_out stored)
                    vector.wait_ge(s_out[io], out_cum_after[g - nbuf_out])
                vector.tensor_tensor(OUT[:, :rd, 0, :, 0], t1, t2, sub).then_inc(
                    s_dve, 1
                )
                vector.tensor_tensor(OUT[:, :rd, 0, :, 1], t1, t2, add).then_inc(
                    s_dve, 1
                )
                vector.wait_ge(s_dve, 8 * g + 4)
                vector.tensor_tensor(OUT[:, :rd, 1, :, 0], s1, s2, sub).then_inc(
                    s_dve, 1
                )
                vector.tensor_tensor(OUT[:, :rd, 1, :, 1], s1, s2, add).then_inc(
                    s_dve, 1
                )

        @block.gpsimd
        def _(pool: bass.BassEngine):
            for g, (b, c0, G) in enumerate(groups):
                if not pool_ops[g]:
                    continue
                i = g % nbuf
                rd, pr = rd_of[g], pr_of[g]
                base = pool_base[g]
                pool.wait_ge(s_ina[i], 32 * (g // nbuf + 1))
                io = g % nbuf_out
                LL, HF, OUT = LLb[i], HFb[i], OUTb[io]
                ll_v = LL[:, rd:G]
                lh_v, hl_v, hh_v = HF[:, 0, rd:G], HF[:, 1, rd:G], HF[:, 2, rd:G]
                t1, t2 = T1p[:, :pr], T2p[:, :pr]
                s1, s2 = S1p[:, :pr], S2p[:, :pr]
                if base >= 8:
                    # WAR: prev active group's a,b ops still read T1p,T2p
                    pool.wait_ge(s_pool, base - 2)
                pool.tensor_tensor(t1, ll_v, lh_v, sub).then_inc(s_pool, 1)
                if base >= 8:
                    pool.wait_ge(s_pool, base)
                pool.tensor_tensor(s1, ll_v, lh_v, add).then_inc(s_pool, 1)
                pool.wait_ge(s_inb[i], 32 * (g // nbuf + 1))
                pool.tensor_tensor(t2, hl_v, hh_v, sub).then_inc(s_pool, 1)
                pool.tensor_tensor(s2, hl_v, hh_v, add).then_inc(s_pool, 1)
                pool.wait_ge(s_pool, base + 3)
                if g >= nbuf_out:
                    pool.wait_ge(s_out[io], out_cum_after[g - nbuf_out])
                pool.tensor_tensor(OUT[:, rd:G, 0, :, 0], t1, t2, sub).then_inc(
                    s_pool, 1
                )
                pool.tensor_tensor(OUT[:, rd:G, 0, :, 1], t1, t2, add).then_inc(
                    s_pool, 1
                )
                pool.wait_ge(s_pool, base + 4)
                pool.tensor_tensor(OUT[:, rd:G, 1, :, 0], s1, s2, sub).then_inc(
                    s_pool, 1
                )
                pool.tensor_tensor(OUT[:, rd:G, 1, :, 1], s1, s2, add).then_inc(
                    s_pool, 1
                )

    return nc


_NC_CACHE = {}


def _get_nc():
    if "nc" not in _NC_CACHE:
        _NC_CACHE["nc"] = build_haar_nc()
    return _NC_CACHE["nc"]


def kernel(ll: np.ndarray, hf: np.ndarray) -> np.ndarray:
    ll = np.ascontiguousarray(ll, dtype=np.float32)
    hf = np.ascontiguousarray(hf, dtype=np.float32)
    nc = _get_nc()
    in_maps = [
        {
            "ll": ll[i * B_LOC : (i + 1) * B_LOC],
            "hf": hf[i * B_LOC : (i + 1) * B_LOC],
        }
        for i in range(N_CORES)
    ]
    res = run_bass_kernel_spmd(nc, in_maps, list(range(N_CORES))).results
    return np.concatenate([res[i]["out"] for i in range(N_CORES)], axis=0)

